# revision 19
# baseline (speedup 1.0000x reference)
"""DeepFM (embedding gather + FM + 5-layer seq-1 attention + head) on 8 trn2 cores.

Strategy: data-parallel over batch (2048 rows/core), gather-paced per-tile
pipeline. The serial wall is SWDGE descriptor generation for the embedding
gathers (624 indirect DMAs x ~1.04us fixed overhead on gpsimd); everything
else is hidden beneath it:
  - bf16 matmul operands everywhere -> all 15 per-layer weight tiles live in
    SBUF for the whole kernel (no streaming, no rotation constraints), full
    PE rate at any free dim, FWL weight loads.
  - per-128-row-tile processing: each tile runs gather -> FM/normalize ->
    transposes -> 5 attention layers -> head while the next tile's gathers
    stream. Tail exposure after the last gather is one tile's compute.
  - algebraic folding (host, f64): U_l = Wo_{l-1} @ Wv_l removes the
    out-projection (15 instead of 20 DxD matmuls); P = Wo_4 @ m3_w replaces
    the m3 head path; s2 == 1 after L2-norm so f2 = 0.5*u^2 - 0.5 via a
    16-wide sum matmul + ACT Square, bias folded into the head.
  - f1 and the ones row ride along in the PE transposes (xt: [xhat|1|f1]).
"""
import numpy as np
import ml_dtypes

import concourse.bass as bass
import concourse.mybir as mybir
from concourse.tile import TileContext
from concourse.vector_clock import ScopedClock
from concourse.masks import make_identity
from concourse.bass_utils import run_bass_kernel_spmd

F32 = mybir.dt.float32
BF16 = mybir.dt.bfloat16
I32 = mybir.dt.int32
AF = mybir.ActivationFunctionType
OP = mybir.AluOpType

# problem constants (hardcoded per contract)
N = 16384
F = 39
V = 100000
E = 16
EW = E + 1          # fused [emb1 | emb2] row width
D = F * E           # 624
L = 5
NCORES = 8
NPC = N // NCORES   # 2048 rows per core
NT = NPC // 128     # 16 tiles of 128 rows
NB = 256            # processing chunk free dim (2 tiles)
NCH = NPC // NB     # 8 chunks
KDIMS = [128, 128, 128, 128, 112]      # K-tiles over 624
KDIMS_AUG = [128, 128, 128, 128, 113]  # incl. ones/bias row at 112 of tail tile
MDIMS = [128, 128, 128, 128, 112]
XT_W = D + 1 + F    # 664: [xhat | ones | f1]
SQRT_HALF = 0.7071067811865476

MAX_WAITS = 1

LAST_RESULT = None  # test harness reads exec_time_ns from here


class SplitWaitTileContext(TileContext):
    """Walrus (CoreV3) accepts at most one sync-wait command per instruction;
    Tile can emit several. Split extras onto preceding same-engine NOPs, and
    do the same for the kernel-tail drain."""

    def _add_instruction(self, inst):
        si = inst.sync_info
        if si is not None and len(si.on_wait) > MAX_WAITS:
            waits = list(si.on_wait)
            head, tail = waits[:-MAX_WAITS], waits[-MAX_WAITS:]
            for i in range(0, len(head), MAX_WAITS):
                nop = mybir.InstNoOp(
                    name=self.nc.get_next_instruction_name(),
                    sync_info=mybir.SyncInfo(
                        on_wait=head[i : i + MAX_WAITS], on_update=[]
                    ),
                    bass_nofuse=True,
                    engine=inst.engine,
                )
                super()._add_instruction(nop)
            inst.sync_info = mybir.SyncInfo(on_wait=tail, on_update=si.on_update)
        super()._add_instruction(inst)

    def _drain_and_barrier(self, tick_clock, wait_clock):
        nc = self.nc
        probe = nc.sync.nop(nofuse=True, hint="tail_wait_probe")
        wait_clock.add_sem_waits(
            probe.ins, ScopedClock({None: tick_clock.global_clock})
        )
        waits = list(probe.ins.sync_info.on_wait)
        probe.ins.sync_info.on_wait = waits[:MAX_WAITS]
        for i in range(MAX_WAITS, len(waits), MAX_WAITS):
            nop = nc.sync.nop(nofuse=True, hint="tail_wait_split")
            nop.ins.sync_info = mybir.SyncInfo(
                on_wait=waits[i : i + MAX_WAITS], on_update=[]
            )
        drain_inst = nc.sync.drain()
        wait_clock.add_sem_waits(
            drain_inst.ins, ScopedClock({None: tick_clock.global_clock})
        )
        if len(drain_inst.ins.sync_info.on_wait) > MAX_WAITS:
            drain_inst.ins.sync_info.on_wait = []
        nc.all_engine_barrier()
        assert self.sems is not None
        popped = nc._tile_sem_poison_stack.pop()
        assert popped is self._sem_poison
        nc.clear_and_free_semaphores(list(self.sems.allocated().values()))
        nc.all_engine_barrier()


def to_bf16(a):
    return np.asarray(a, dtype=np.float64).astype(ml_dtypes.bfloat16)


def build_nc():
    nc = bass.Bass()

    tab = nc.declare_dram_parameter("tab", [F * V, EW], F32, isOutput=False)
    idx = nc.declare_dram_parameter("idx", [NT, 128, F], I32, isOutput=False)
    xvd = nc.declare_dram_parameter("xvd", [NT, 128, F], F32, isOutput=False)
    wq = nc.declare_dram_parameter("wq", [L, 128, 5 * D], BF16, isOutput=False)
    wk = nc.declare_dram_parameter("wk", [L, 128, 5 * D], BF16, isOutput=False)
    wu = nc.declare_dram_parameter("wu", [L, 128, 5 * D], BF16, isOutput=False)
    am = nc.declare_dram_parameter("am", [128, 5 * F], BF16, isOutput=False)
    bm = nc.declare_dram_parameter("bm", [F, 5 * 128], BF16, isOutput=False)
    s16 = nc.declare_dram_parameter("s16", [128, 5 * E], BF16, isOutput=False)
    m12 = nc.declare_dram_parameter("m12", [80, 9], BF16, isOutput=False)
    m3p = nc.declare_dram_parameter("m3p", [128, 5 * 4], BF16, isOutput=False)
    ffw9 = nc.declare_dram_parameter("ffw9", [9, 13], BF16, isOutput=False)
    ffw3 = nc.declare_dram_parameter("ffw3", [4, 13], BF16, isOutput=False)
    fdw = nc.declare_dram_parameter("fdw", [13, 2], BF16, isOutput=False)
    y = nc.declare_dram_parameter("y", [NPC, 2], F32, isOutput=True)

    with SplitWaitTileContext(nc) as tc:
        with (
            tc.tile_pool(name="const", bufs=1) as cp,
            tc.tile_pool(name="persist", bufs=1) as pp,
            tc.tile_pool(name="work", bufs=2) as wk_p,
            tc.tile_pool(name="small", bufs=2) as sp,
            tc.tile_pool(name="psA", bufs=2, space="PSUM") as ps_qk,
            tc.tile_pool(name="psB", bufs=1, space="PSUM") as ps_s,
            tc.tile_pool(name="psC", bufs=2, space="PSUM") as ps_v,
            tc.tile_pool(name="psD", bufs=1, space="PSUM") as ps_b,
            tc.tile_pool(name="psTA", bufs=1, space="PSUM") as ps_ta,
            tc.tile_pool(name="psTB", bufs=1, space="PSUM") as ps_tb,
        ):
            ident = cp.tile([128, 128], F32)
            make_identity(nc, ident[:])

            XVT = [
                pp.tile([128, NPC], BF16, tag=f"xvt{i}", name=f"xvt{i}")
                for i in range(5)
            ]
            ATT = [
                pp.tile([128, NPC], BF16, tag=f"att{i}", name=f"att{i}")
                for i in range(5)
            ]
            # rows: 0 ones, 1-39 f1T, 40-63 unused (partition-base alignment),
            # 64-79 f2p
            ft = pp.tile([80, NPC], BF16, tag="ft")
            out_sb = pp.tile([128, NT * 2], F32, tag="outsb")

            # ones row (112) for the augmented bias path of layers >= 1;
            # engines need partition base in {0,32,64,96}, so set 96:128 —
            # rows 96-111 are overwritten by chain() before any read.
            nc.vector.memset(ATT[4][96:128, :], 1.0)

            # ---- gather issue for one tile (keep gpsimd queue pure) ----
            def gather_tile(t):
                idx_t = sp.tile([128, F], I32, tag="idx", bufs=3)
                nc.sync.dma_start(out=idx_t[:], in_=idx[t, :, :])
                xv_t = sp.tile([128, F], F32, tag="xv", bufs=3)
                nc.sync.dma_start(out=xv_t[:], in_=xvd[t, :, :])
                g = wk_p.tile([128, F * EW], F32, tag="g", bufs=3)
                for f in range(F):
                    nc.gpsimd.indirect_dma_start(
                        out=g[:, f * EW : (f + 1) * EW],
                        out_offset=None,
                        in_=tab[:],
                        in_offset=bass.IndirectOffsetOnAxis(
                            ap=idx_t[:, f : f + 1], axis=0
                        ),
                    )
                return g, xv_t

            # first two tiles' gathers go ahead of the weight DMAs so the
            # gather stream (the serial wall) starts immediately
            pend = {0: gather_tile(0), 1: gather_tile(1)}

            # ---- weights for all layers, resident whole-kernel ----
            WQ, WK, WU = [], [], []
            for l in range(L):
                t_ = cp.tile([128, 5 * D], BF16, name=f"wq{l}")
                nc.sync.dma_start(out=t_[:], in_=wq[l, :, :])
                WQ.append(t_)
                t_ = cp.tile([128, 5 * D], BF16, name=f"wk{l}")
                nc.sync.dma_start(out=t_[:], in_=wk[l, :, :])
                WK.append(t_)
                t_ = cp.tile([128, 5 * D], BF16, name=f"wu{l}")
                nc.sync.dma_start(out=t_[:], in_=wu[l, :, :])
                WU.append(t_)

            a_sb = cp.tile([128, 5 * F], BF16)
            nc.sync.dma_start(out=a_sb[:], in_=am[:])
            b_sb = cp.tile([F, 5 * 128], BF16)
            nc.sync.dma_start(out=b_sb[:], in_=bm[:])
            s16_sb = cp.tile([128, 5 * E], BF16)
            nc.sync.dma_start(out=s16_sb[:], in_=s16[:])
            m12_sb = cp.tile([80, 9], BF16)
            nc.sync.dma_start(out=m12_sb[:], in_=m12[:])
            m3p_sb = cp.tile([128, 5 * 4], BF16)
            nc.sync.dma_start(out=m3p_sb[:], in_=m3p[:])
            ffw9_sb = cp.tile([9, 13], BF16)
            nc.sync.dma_start(out=ffw9_sb[:], in_=ffw9[:])
            ffw3_sb = cp.tile([4, 13], BF16)
            nc.sync.dma_start(out=ffw3_sb[:], in_=ffw3[:])
            fdw_sb = cp.tile([13, 2], BF16)
            nc.sync.dma_start(out=fdw_sb[:], in_=fdw[:])

            # ---- compute half of phase A: FM prep + transposes ----
            def compute_tile(t, g, xv_t):
                g3 = g[:].rearrange("p (f j) -> p f j", j=EW)
                # scale by Xv (also scales the emb1 slot -> f1)
                nc.vector.tensor_tensor(
                    out=g3,
                    in0=g3,
                    in1=xv_t[:].unsqueeze(2).to_broadcast([128, F, EW]),
                    op=OP.mult,
                )
                g_xv = g3[:, :, 1:]                      # (p, f, e)
                g_ef = g_xv.transpose([0, 2, 1])         # (p, e, f) view
                xt = wk_p.tile([128, XT_W], F32, tag="xt")
                sq_v = xt[:, :D].rearrange("p (e f) -> p e f", f=F)
                nc.scalar.activation(out=sq_v, in_=g_ef, func=AF.Square)
                ss = sp.tile([128, 16], F32, tag="ss")
                nc.vector.reduce_sum(out=ss[:], in_=sq_v, axis=mybir.AxisListType.X)
                mx = sp.tile([128, 16], F32, tag="mx")
                nc.vector.tensor_scalar_max(out=mx[:], in0=ss[:], scalar1=1e-24)
                rt = sp.tile([128, 16], F32, tag="rt")
                nc.scalar.sqrt(out=rt[:], in_=mx[:])
                inv = sp.tile([128, 16], F32, tag="inv")
                nc.vector.reciprocal(out=inv[:], in_=rt[:])
                # normalized xv in (f, e) = d layout — overwrites sq storage
                xv_v = xt[:, :D].rearrange("p (f e) -> p f e", e=E)
                nc.vector.tensor_tensor(
                    out=xv_v,
                    in0=g_xv,
                    in1=inv[:].unsqueeze(1).to_broadcast([128, F, E]),
                    op=OP.mult,
                )
                nc.vector.memset(xt[:, D : D + 1], 1.0)
                nc.scalar.activation(
                    out=xt[:, D + 1 : XT_W], in_=g3[:, :, 0], func=AF.Copy
                )
                # transposes to feature-major: 5x128 cols -> XVT, 24 -> ft
                ts = slice(t * 128, (t + 1) * 128)
                for cc in range(5):
                    pt = ps_ta.tile([128, 128], F32, tag="pta")
                    nc.tensor.transpose(
                        out=pt[:, :],
                        in_=xt[:, cc * 128 : (cc + 1) * 128],
                        identity=ident[:],
                    )
                    if cc % 2 == 0:
                        nc.vector.tensor_copy(out=XVT[cc][:, ts], in_=pt[:, :])
                    else:
                        nc.scalar.activation(
                            out=XVT[cc][:, ts], in_=pt[:, :], func=AF.Copy
                        )
                pt = ps_ta.tile([128, 128], F32, tag="pta")
                nc.tensor.transpose(
                    out=pt[:40, :], in_=xt[:, D:XT_W], identity=ident[:]
                )
                nc.scalar.activation(
                    out=ft[0:40, ts], in_=pt[:40, :], func=AF.Copy
                )

            # ---- scores for (l, chunk): s_sb (39, w) bf16 ----
            def scores(l, cs, w):
                wq_t, wk_t = WQ[l], WK[l]
                pss = ps_s.tile([F, NB], F32, tag="ss")
                pend = None

                def issue_pss(m, p_sb, mw):
                    nc.tensor.matmul(
                        out=pss[:, :w],
                        lhsT=a_sb[:mw, m * F : (m + 1) * F],
                        rhs=p_sb[:mw, :w],
                        start=(m == 0),
                        stop=(m == 4),
                    )

                for m in range(5):
                    mw = MDIMS[m]
                    psq = ps_qk.tile([128, NB], F32, tag="qk")
                    for kb in range(5):
                        kw = KDIMS_AUG[kb]
                        nc.tensor.matmul(
                            out=psq[:mw, :w],
                            lhsT=wq_t[:kw, kb * D + m * 128 : kb * D + m * 128 + mw],
                            rhs=XVT[kb][:kw, cs],
                            start=(kb == 0),
                            stop=(kb == 4),
                        )
                    q_sb = wk_p.tile([128, NB], F32, tag="qsb")
                    nc.scalar.activation(
                        out=q_sb[:mw, :w], in_=psq[:mw, :w], func=AF.Copy
                    )
                    psk = ps_qk.tile([128, NB], F32, tag="qk")
                    for kb in range(5):
                        kw = KDIMS_AUG[kb]
                        nc.tensor.matmul(
                            out=psk[:mw, :w],
                            lhsT=wk_t[:kw, kb * D + m * 128 : kb * D + m * 128 + mw],
                            rhs=XVT[kb][:kw, cs],
                            start=(kb == 0),
                            stop=(kb == 4),
                        )
                    if pend is not None:
                        issue_pss(*pend)
                    p_sb = wk_p.tile([128, NB], BF16, tag="psb")
                    nc.vector.tensor_tensor(
                        out=p_sb[:mw, :w],
                        in0=q_sb[:mw, :w],
                        in1=psk[:mw, :w],
                        op=OP.mult,
                    )
                    pend = (m, p_sb, mw)
                issue_pss(*pend)
                s_sb = wk_p.tile([F, NB], BF16, tag="ssb")
                nc.vector.tensor_copy(out=s_sb[:, :w], in_=pss[:, :w])
                return s_sb

            # ---- chain for (l, t): v = att_prev @ U_l (+bias); att = s * v ----
            def chain(l, cs, w, s_sb):
                wu_t = WU[l]
                Xsrc = XVT if l == 0 else ATT
                for m in range(5):
                    mw = MDIMS[m]
                    psv = ps_v.tile([128, NB], F32, tag="v")
                    for kb in range(5):
                        kw = KDIMS_AUG[kb]
                        nc.tensor.matmul(
                            out=psv[:mw, :w],
                            lhsT=wu_t[:kw, kb * D + m * 128 : kb * D + m * 128 + mw],
                            rhs=Xsrc[kb][:kw, cs],
                            start=(kb == 0),
                            stop=(kb == 4),
                        )
                    v_sb = wk_p.tile([128, NB], F32, tag="vsb")
                    nc.scalar.activation(
                        out=v_sb[:mw, :w], in_=psv[:mw, :w], func=AF.Copy
                    )
                    psb = ps_b.tile([128, NB], F32, tag="b")
                    nc.tensor.matmul(
                        out=psb[:mw, :w],
                        lhsT=b_sb[:, m * 128 : m * 128 + mw],
                        rhs=s_sb[:, :w],
                        start=True,
                        stop=True,
                    )
                    nc.vector.tensor_tensor(
                        out=ATT[m][:mw, cs],
                        in0=v_sb[:mw, :w],
                        in1=psb[:mw, :w],
                        op=OP.mult,
                    )

            # f2p = 0.5*u^2 where u = sum_f xhat (uses XVT only)
            def f2_chunk(cs, w):
                psu = ps_tb.tile([16, NB], F32, tag="ptb")
                for kb in range(5):
                    kw = KDIMS[kb]
                    nc.tensor.matmul(
                        out=psu[:, :w],
                        lhsT=s16_sb[:kw, kb * E : (kb + 1) * E],
                        rhs=XVT[kb][:kw, cs],
                        start=(kb == 0),
                        stop=(kb == 4),
                    )
                nc.scalar.activation(
                    out=ft[64:80, cs], in_=psu[:, :w], func=AF.Square,
                    scale=SQRT_HALF,
                )

            def head(t0, ntile, cs, w):
                pff = ps_tb.tile([9, NB], F32, tag="ptb")
                nc.tensor.matmul(
                    out=pff[:, :w],
                    lhsT=m12_sb[0:40, :],
                    rhs=ft[0:40, cs],
                    start=True,
                    stop=False,
                )
                nc.tensor.matmul(
                    out=pff[:, :w],
                    lhsT=m12_sb[64:80, :],
                    rhs=ft[64:80, cs],
                    start=False,
                    stop=True,
                )
                ffin12 = wk_p.tile([9, NB], BF16, tag="ffin12")
                nc.vector.tensor_copy(out=ffin12[:, :w], in_=pff[:, :w])
                ph3 = ps_tb.tile([4, NB], F32, tag="ptb")
                for kb in range(5):
                    kw = KDIMS_AUG[kb]
                    nc.tensor.matmul(
                        out=ph3[:, :w],
                        lhsT=m3p_sb[:kw, kb * 4 : (kb + 1) * 4],
                        rhs=ATT[kb][:kw, cs],
                        start=(kb == 0),
                        stop=(kb == 4),
                    )
                ffin3 = wk_p.tile([4, NB], BF16, tag="ffin3")
                nc.vector.tensor_copy(out=ffin3[:, :w], in_=ph3[:, :w])
                pf = ps_tb.tile([13, NB], F32, tag="ptb")
                nc.tensor.matmul(
                    out=pf[:, :w], lhsT=ffw9_sb[:], rhs=ffin12[:, :w],
                    start=True, stop=False,
                )
                nc.tensor.matmul(
                    out=pf[:, :w], lhsT=ffw3_sb[:], rhs=ffin3[:, :w],
                    start=False, stop=True,
                )
                ffout = wk_p.tile([13, NB], BF16, tag="ffout")
                nc.vector.tensor_scalar_max(
                    out=ffout[:, :w], in0=pf[:, :w], scalar1=0.0
                )
                for q in range(ntile):
                    tt_ = t0 + q
                    ptot = ps_tb.tile([128, 2], F32, tag="ptb")
                    nc.tensor.matmul(
                        out=ptot[:, :],
                        lhsT=ffout[:, q * 128 : (q + 1) * 128],
                        rhs=fdw_sb[:],
                        start=True,
                        stop=True,
                    )
                    nc.vector.tensor_copy(
                        out=out_sb[:, tt_ * 2 : (tt_ + 1) * 2], in_=ptot[:, :]
                    )

            # ---- gather-paced pipeline: 2-tile chunks, then single-tile
            # chunks at the end so the post-gather tail is one tile's work
            chunks = [(0, 2), (2, 2), (4, 2), (6, 2), (8, 2), (10, 2),
                      (12, 2), (14, 1), (15, 1)]
            tnext = 2
            for (t0, ntile) in chunks:
                for t in range(t0, t0 + ntile):
                    if tnext < NT:
                        pend[tnext] = gather_tile(tnext)
                        tnext += 1
                    compute_tile(t, *pend.pop(t))
                cs = slice(t0 * 128, (t0 + ntile) * 128)
                w = ntile * 128
                for l in range(L):
                    s_sb = scores(l, cs, w)
                    if l == 0:
                        f2_chunk(cs, w)
                    chain(l, cs, w, s_sb)
                head(t0, ntile, cs, w)

            # final store: out_sb (128, NT, 2) -> y (NT*128, 2)
            nc.sync.dma_start(
                out=y[:].rearrange("(t p) j -> p t j", p=128),
                in_=out_sb[:].rearrange("p (t j) -> p t j", j=2),
            )

    return nc


def host_pack(Xi, Xv, emb1, emb2, Wq, bq, Wk, bk, Wv, bv, Wo, bo,
              m1_w, m1_b, m2_w, m2_b, m3_w, m3_b, ffw_w, ffw_b, fd_w, fd_b):
    """Preprocess full inputs into per-core input maps."""
    idxg = (
        np.arange(F, dtype=np.int64)[None, :] * V + np.asarray(Xi)[:, :, 0]
    ).astype(np.int32)                                    # (N, F)
    Xv = np.asarray(Xv, dtype=np.float32)
    tab = np.concatenate(
        [np.asarray(emb1).reshape(F * V, 1), np.asarray(emb2).reshape(F * V, E)],
        axis=1,
    ).astype(np.float32)                                  # (F*V, 17)

    def pack_w(Wx, bx):
        # (D, D)+(D,) -> (128, 5, D) lhsT tiles, bias at row 112 of kb=4
        out = np.zeros((128, 5, D), dtype=np.float64)
        for kb in range(5):
            kw = KDIMS[kb]
            out[:kw, kb, :] = Wx[kb * 128 : kb * 128 + kw, :]
        out[112, 4, :] = bx
        return to_bf16(out).reshape(128, 5 * D)

    Wq = np.asarray(Wq, dtype=np.float64)
    Wk = np.asarray(Wk, dtype=np.float64)
    Wv = np.asarray(Wv, dtype=np.float64)
    Wo = np.asarray(Wo, dtype=np.float64)
    bq_ = np.asarray(bq, dtype=np.float64)
    bk_ = np.asarray(bk, dtype=np.float64)
    bv_ = np.asarray(bv, dtype=np.float64)
    bo_ = np.asarray(bo, dtype=np.float64)

    wq_h = np.stack([pack_w(Wq[l], bq_[l]) for l in range(L)])
    wk_h = np.stack([pack_w(Wk[l], bk_[l]) for l in range(L)])
    # folded U_l = Wo_{l-1} @ Wv_l (U_0 = Wv_0); c_l = bo_{l-1} @ Wv_l + bv_l
    wu_h = []
    for l in range(L):
        if l == 0:
            U, cvec = Wv[0], bv_[0]
        else:
            U = Wo[l - 1] @ Wv[l]
            cvec = bo_[l - 1] @ Wv[l] + bv_[l]
        wu_h.append(pack_w(U, cvec))
    wu_h = np.stack(wu_h)

    am_h = np.zeros((128, 5, F), dtype=np.float64)
    for kb in range(5):
        for p in range(KDIMS[kb]):
            d = kb * 128 + p
            am_h[p, kb, d // 16] = 0.25
    am_h = to_bf16(am_h).reshape(128, 5 * F)

    bm_h = np.zeros((F, 5, 128), dtype=np.float64)
    for m in range(5):
        for p in range(MDIMS[m]):
            d = m * 128 + p
            bm_h[d // 16, m, p] = 1.0
    bm_h = to_bf16(bm_h).reshape(F, 5 * 128)

    s16_h = np.zeros((128, 5, E), dtype=np.float64)
    for kb in range(5):
        for p in range(KDIMS[kb]):
            d = kb * 128 + p
            s16_h[p, kb, d % 16] = 1.0
    s16_h = to_bf16(s16_h).reshape(128, 5 * E)

    m1_w = np.asarray(m1_w, dtype=np.float64)
    m1_b = np.asarray(m1_b, dtype=np.float64)
    m2_w = np.asarray(m2_w, dtype=np.float64)
    m2_b = np.asarray(m2_b, dtype=np.float64)
    # ft rows: 0 ones, 1-39 f1, 40-55 f2p; the -0.5 of f2 = f2p - 0.5 and
    # m2_b fold into the ones row
    m12_h = np.zeros((80, 9), dtype=np.float64)
    m12_h[0, 0:4] = m1_b
    m12_h[0, 4:8] = m2_b - 0.5 * m2_w.sum(axis=0)
    m12_h[0, 8] = 1.0
    m12_h[1:40, 0:4] = m1_w
    m12_h[64:80, 4:8] = m2_w
    m12_h = to_bf16(m12_h)

    # folded P = Wo_4 @ m3_w, bias = bo_4 @ m3_w + m3_b
    m3_w = np.asarray(m3_w, dtype=np.float64)
    m3_b = np.asarray(m3_b, dtype=np.float64)
    P = Wo[4] @ m3_w
    dvec = bo_[4] @ m3_w + m3_b
    m3p_h = np.zeros((128, 5, 4), dtype=np.float64)
    for kb in range(5):
        kw = KDIMS[kb]
        m3p_h[:kw, kb, :] = P[kb * 128 : kb * 128 + kw, :]
    m3p_h[112, 4, :] = dvec
    m3p_h = to_bf16(m3p_h).reshape(128, 5 * 4)

    ffw_w = np.asarray(ffw_w, dtype=np.float64)
    ffw9_h = np.zeros((9, 13), dtype=np.float64)
    ffw9_h[0:8, 0:12] = ffw_w[0:8]
    ffw9_h[8, 0:12] = np.asarray(ffw_b, dtype=np.float64)
    ffw9_h[8, 12] = 1.0   # ones-row producer (ffin12 row 8 is all-ones)
    ffw9_h = to_bf16(ffw9_h)
    ffw3_h = np.zeros((4, 13), dtype=np.float64)
    ffw3_h[:, 0:12] = ffw_w[8:12]
    ffw3_h = to_bf16(ffw3_h)

    fdw_h = np.zeros((13, 2), dtype=np.float64)
    fdw_h[:12] = np.asarray(fd_w, dtype=np.float64)
    fdw_h[12] = np.asarray(fd_b, dtype=np.float64)
    fdw_h = to_bf16(fdw_h)

    in_maps = []
    for core in range(NCORES):
        sl = slice(core * NPC, (core + 1) * NPC)
        idx_r = idxg[sl].reshape(NT, 128, F)
        xv_r = Xv[sl].reshape(NT, 128, F)
        in_maps.append(
            dict(
                tab=tab,
                idx=np.ascontiguousarray(idx_r),
                xvd=np.ascontiguousarray(xv_r),
                wq=wq_h, wk=wk_h, wu=wu_h,
                am=am_h, bm=bm_h, s16=s16_h,
                m12=m12_h, m3p=m3p_h,
                ffw9=ffw9_h, ffw3=ffw3_h, fdw=fdw_h,
            )
        )
    return in_maps


_NC_CACHE = None


def kernel(**inputs):
    global _NC_CACHE, LAST_RESULT
    in_maps = host_pack(**inputs)
    if _NC_CACHE is None:
        _NC_CACHE = build_nc()
    res = run_bass_kernel_spmd(_NC_CACHE, in_maps, list(range(NCORES)))
    LAST_RESULT = res
    out = np.concatenate([res.results[c]["y"] for c in range(NCORES)], axis=0)
    return out


if __name__ == "__main__":
    print("building...")
    nc = build_nc()
    print("built ok")
